# revision 32
# baseline (speedup 1.0000x reference)
"""Gated attention-with-pair-bias kernel for 8 Trainium2 NeuronCores.

Problem: B=2, Q=K=2048, C=256, H=8 heads, D=32 per head.
  q = (q_x @ Wq.T)/sqrt(D); k = kv_x @ Wk.T; v = kv_x @ Wv.T   (per head h)
  S = q @ k.T + bias_mask + bias_pair; w = softmax_k(S)
  o = (w @ v) * sigmoid(q_x @ Wg.T + bg); out = o @ Wo.T + bo

Sharding: one head per core (8 heads / 8 cores); each core handles both
batch elements.  Per-head output-projection partials are normalized by
the softmax denominator and summed on host (1/l commutes past Wo).

On-chip layout is "ST" (scores transposed): S.T tiles are [k->128
partitions, q->512 free].  The engine budget is dominated by the Act
(ScalarE) exp stream -- 8.4M score elements/core at 1 elem/cycle/lane is
a ~70us floor -- so every other engine is kept strictly below it:
  - score matmuls: D=32 contraction packed 4x with PE row tiling
    (tile_position); the 4 strip-MMs execute concurrently (~1 MM's time).
  - bias_pair add is split: strips 0,1 of each group get it via an
    identity-matmul PSUM pre-fill (PE has slack), strips 2,3 get it
    after the exp as a bf16 2x-mode DVE multiply with exp(bias_pair)
    precomputed on host: exp(s+b) = exp(s)*exp(b).
  - bias_mask folds into v (and into the appended ones-column that
    yields the softmax denominator from the o-matmul).
  - gates use tanh (same Act table-set as exp; sigmoid would force two
    ~2.7us table switches): sigmoid(z) = 0.5*tanh(z/2)+0.5.
  - everything HBM-side is bf16; bias_pair is restaged on host into
    fully-contiguous 512KB tiles so DMA runs at line rate.
"""

import math
import sys

sys.path.insert(0, "/opt/trn_rl_repo")

import numpy as np

H, D, B, Q, K, C = 8, 32, 2, 2048, 2048, 256
NQC = 4          # q chunks of 512
NKT = K // 128   # 16 k tiles
# which of the 4 strips per group take the identity-MM bias path (True)
# vs the post-exp multiply path (False)
A_OF4 = (True, True, False, False)

_CACHE = {}


def _build():
    import concourse.bacc as bacc
    import concourse.mybir as mybir
    from concourse.tile import TileContext

    F32 = mybir.dt.float32
    BF16 = mybir.dt.bfloat16
    EXP = mybir.ActivationFunctionType.Exp
    TANH = mybir.ActivationFunctionType.Tanh
    MULT = mybir.AluOpType.mult
    ADD = mybir.AluOpType.add

    nc = bacc.Bacc(None, target_bir_lowering=False)
    qxT = nc.dram_tensor("qxT", [B, 2, 128, Q], BF16, kind="ExternalInput")
    kvT = nc.dram_tensor("kvT", [B, 2, 128, K], BF16, kind="ExternalInput")
    # bias tiles: [qc, g, p, j, q']; j in {0,1}: raw bias (id-MM path),
    # j in {2,3}: exp(bias) (post-exp multiply path)
    bpt = nc.dram_tensor("bpt", [NQC, 4, 128, 4, 512], BF16, kind="ExternalInput")
    # all bf16 weights/constants in one blob -> one DMA on the sync queue
    # cols: wq 2x128 | wk 8x128 | id 128 | wv 2x32 | wg 2x32 | expbmb 2x16
    WPK = 1568
    wpk = nc.dram_tensor("wpk", [128, WPK], BF16, kind="ExternalInput")
    expbm = nc.dram_tensor("expbm", [B, 128, NKT], F32, kind="ExternalInput")
    bgv = nc.dram_tensor("bgv", [D, 1], F32, kind="ExternalInput")
    ogout = nc.dram_tensor("ogout", [B, NQC, 32, 512], BF16, kind="ExternalOutput")
    lout = nc.dram_tensor("lout", [B, NQC, 512], F32, kind="ExternalOutput")

    with TileContext(nc) as tc:
        with (
            tc.tile_pool(name="ld", bufs=1) as ld,
            tc.tile_pool(name="pers", bufs=1) as pers,
            tc.tile_pool(name="bp", bufs=8) as bppool,
            tc.tile_pool(name="wp", bufs=3) as wpool,
            tc.tile_pool(name="ob", bufs=2) as obpool,
            tc.tile_pool(name="ps_sc", bufs=2, space="PSUM") as ps_sc,
            tc.tile_pool(name="ps_o", bufs=2, space="PSUM") as ps_o,
            tc.tile_pool(name="ps_m", bufs=2, space="PSUM") as ps_m,
        ):
            # ---- one packed-weight DMA, then per-batch activations, so
            # the first projection matmuls start as early as possible ----
            # warm the Act table set (exp/tanh) off the critical path
            dummy = pers.tile([1, 2], F32, name="dummy")
            nc.vector.memset(dummy[:, :], 0.0)
            nc.scalar.activation(dummy[:, 0:1], dummy[:, 1:2], EXP)

            wpk_sb = pers.tile([128, 1568], BF16, name="wpk_sb")
            nc.sync.dma_start(out=wpk_sb[:, :], in_=wpk[:, :])
            wq_sb = [wpk_sb[:, ch * 128:(ch + 1) * 128] for ch in range(2)]
            wk_sb = [[wpk_sb[:, 256 + (j * 2 + ch) * 128:256 + (j * 2 + ch + 1) * 128]
                      for ch in range(2)] for j in range(4)]
            id_sb = wpk_sb[:, 1280:1408]
            wv_sb = [wpk_sb[:, 1408 + ch * 32:1408 + (ch + 1) * 32] for ch in range(2)]
            wg_sb = [wpk_sb[:, 1472 + ch * 32:1472 + (ch + 1) * 32] for ch in range(2)]
            bmc_sb = [wpk_sb[:, 1536 + b * 16:1536 + (b + 1) * 16] for b in range(B)]

            qx_all, kv_all = [], []

            def load_acts(b):
                for ch in range(2):
                    t = ld.tile([128, Q], BF16, name=f"qx{b}{ch}", tag=f"qx{b}{ch}")
                    nc.sync.dma_start(out=t[:, :], in_=qxT[b, ch, :, :])
                    qx_all.append(t)
                    t = ld.tile([128, K], BF16, name=f"kv{b}{ch}", tag=f"kv{b}{ch}")
                    nc.sync.dma_start(out=t[:, :], in_=kvT[b, ch, :, :])
                    kv_all.append(t)

            load_acts(0)
            bm_sb = []
            for b in range(B):
                t = pers.tile([128, NKT], F32, name=f"bm_sb{b}")
                nc.sync.dma_start(out=t[:, :], in_=expbm[b, :, :])
                bm_sb.append(t)
            bg_sb = pers.tile([D, 1], F32, name="bg_sb")
            nc.sync.dma_start(out=bg_sb[:, :], in_=bgv[:, :])
            # qc0's bias tiles BEFORE the b=1 activations: the first exp
            # is gated on bias arrival, b=1 isn't needed until ~35us
            bts0 = []
            for g in range(4):
                t = bppool.tile([128, 4, 512], BF16, tag="bp", name=f"bp0_{g}")
                nc.sync.dma_start(out=t[:, :, :], in_=bpt[0, g])
                bts0.append(t)
            load_acts(1)

            # ---- per-batch projections ----
            # b=0 emits up front; b=1 is deferred into the qc0/b0 compute
            # window so its DMA waits hide under main-loop work
            qT_rep, kT_sb, v_sb, g_sb = [], [], [], []

            def emit_prologue(b):
                qx_b = qx_all[2 * b:2 * b + 2]
                kv_b = kv_all[2 * b:2 * b + 2]
                qT = pers.tile([128, Q], BF16, name=f"qT{b}")
                kT = pers.tile([128, 512], BF16, name=f"kT{b}")
                # [d 0:32 | expbm 32 | zeros 33:64]: 64-wide so pairs of
                # k-tiles run as two concurrent column-tiled o-matmuls
                vt = pers.tile([128, NKT, 64], BF16, name=f"v{b}")
                nc.vector.memset(vt[:, :, :], 0.0)
                gs = pers.tile([32, Q], F32, name=f"g{b}")
                qT_rep.append(qT); kT_sb.append(kT); v_sb.append(vt); g_sb.append(gs)

                # q.T replicated into 4 partition strips via 4x-duplicated
                # weight columns (host-prepared) -> plain M=128 matmuls
                for qc in range(NQC):
                    ps = ps_sc.tile([128, 512], F32, tag="sc", name=f"pq{b}{qc}",
                                    padded_shape=[128, 1024])
                    for ch in range(2):
                        nc.tensor.matmul(
                            ps[:, :], wq_sb[ch][:, :],
                            qx_b[ch][:, qc * 512:(qc + 1) * 512],
                            start=(ch == 0), stop=(ch == 1))
                    nc.vector.tensor_copy(qT[:, qc * 512:(qc + 1) * 512], ps[:, :])

                # k.T in strip layout (strip j holds k tiles {4g+j : g}) via
                # zero-padded strip weights accumulating into one bank
                ps = ps_sc.tile([128, 512], F32, tag="sc", name=f"pk{b}",
                                padded_shape=[128, 1024])
                for j in range(4):
                    kv_r = [t.rearrange("p (g j i) -> p j g i", g=4, i=128)
                            for t in kv_b]
                    for ch in range(2):
                        nc.tensor.matmul(
                            ps[:, :], wk_sb[j][ch][:, :], kv_r[ch][:, j],
                            start=(j == 0 and ch == 0), stop=(j == 3 and ch == 1))
                nc.vector.tensor_copy(kT[:, :], ps[:, :])

                # v in [k-partitions, d] layout; the bias_mask fold
                # (exp(bm) per k row) rides the PSUM evacuation, and the
                # appended ones-column IS exp(bm).  Per-kt psum tiles keep
                # PE-writes and DVE-reads on separate banks (pool
                # rotation); the ones-column goes through a contiguous
                # SBUF staging tile so every vt write is an engine op
                # with solid dependency tracking.
                for kt in range(NKT):
                    pv = ps_m.tile([128, 32], F32, tag="m", name=f"pv{b}{kt}",
                                   padded_shape=[128, 512])
                    for ch in range(2):
                        nc.tensor.matmul(
                            pv[:, :], kv_b[ch][:, kt * 128:(kt + 1) * 128],
                            wv_sb[ch][:, :], start=(ch == 0), stop=(ch == 1))
                    nc.vector.tensor_scalar_mul(vt[:, kt, 0:32], pv[:, :],
                                                bm_sb[b][:, kt:kt + 1])
                nc.vector.tensor_copy(vt[:, :, 32], bmc_sb[b][:, :])

                # gate = sigmoid(Wg_h @ qx.T + bg) = 0.5*tanh((Wg@qx+bg)/2)+0.5
                # tanh shares the exp table-set; the +0.5 scale/offset is DVE
                for qc in range(NQC):
                    pg = ps_m.tile([32, 512], F32, tag="m", name=f"pg{b}{qc}",
                                   padded_shape=[128, 512])
                    for ch in range(2):
                        nc.tensor.matmul(
                            pg[:, :], wg_sb[ch][:, :],
                            qx_b[ch][:, qc * 512:(qc + 1) * 512],
                            start=(ch == 0), stop=(ch == 1))
                    nc.scalar.activation(gs[:, qc * 512:(qc + 1) * 512], pg[:, :],
                                         TANH, bias=bg_sb[:, :], scale=0.5)
                nc.vector.tensor_scalar(gs[:, :], gs[:, :], 0.5, 0.5,
                                        op0=MULT, op1=mybir.AluOpType.add)

            emit_prologue(0)

            # ---- main attention loop ----
            pending = None
            pending_o = None
            for qc in range(NQC):
                if qc == 0:
                    bts = bts0
                else:
                    bts = []
                    for g in range(4):
                        # [128, 4, 512]: one group's (4 k-tiles) bias for
                        # this q chunk, host-restaged fully contiguous
                        t = bppool.tile([128, 4, 512], BF16, tag="bp",
                                        name=f"bp{qc}_{g}")
                        nc.sync.dma_start(out=t[:, :, :], in_=bpt[qc, g])
                        bts.append(t)
                for b in range(B):
                    if qc == 0 and b == 1:
                        emit_prologue(1)
                    po = ps_o.tile([128, 512], F32, tag="o", name=f"po{qc}{b}")
                    for g in range(4):
                        # strips 0,1 (psA): bias via identity-MM PSUM
                        # pre-fill; strips 2,3 (psB): plain scores, bias
                        # multiplied in after the exp
                        psA = ps_sc.tile([128, 2, 512], F32, tag="sc",
                                         name=f"sA{qc}{b}{g}")
                        psB = ps_sc.tile([128, 2, 512], F32, tag="sc",
                                         name=f"sB{qc}{b}{g}")
                        for j in range(4):
                            if A_OF4[j]:
                                ps, jj = (psA, j) if j < 2 else (psB, j - 2)
                                nc.tensor.matmul(
                                    ps[:, jj, :], id_sb[:, :], bts[g][:, j, :],
                                    start=True, stop=False)
                        for j in range(4):
                            ps, jj = (psA, j) if j < 2 else (psB, j - 2)
                            nc.tensor.matmul(
                                ps[:, jj, :],
                                kT_sb[b][32 * j:32 * j + 32, g * 128:(g + 1) * 128],
                                qT_rep[b][32 * j:32 * j + 32, qc * 512:(qc + 1) * 512],
                                start=not A_OF4[j], stop=True,
                                tile_position=(32 * j, 0))
                        wt = wpool.tile([128, 4, 512], BF16, tag="w",
                                        name=f"w{qc}{b}{g}")
                        for ti, ps in ((0, psA), (1, psB)):
                            sl = slice(2 * ti, 2 * ti + 2)
                            if A_OF4[2 * ti] and A_OF4[2 * ti + 1]:
                                nc.scalar.activation(wt[:, sl, :], ps[:, :, :], EXP)
                            else:
                                # exp(s+b) = exp(s)*exp(b): bf16 2x DVE mult
                                wtmp = wpool.tile([128, 2, 512], BF16, tag="wtmp",
                                                  name=f"wtmp{qc}{b}{g}{ti}", bufs=2)
                                nc.scalar.activation(wtmp[:, :, :], ps[:, :, :], EXP)
                                nc.vector.tensor_tensor(wt[:, sl, :], wtmp[:, :, :],
                                                        bts[g][:, sl, :], op=MULT)
                        # o-MMs run one group behind their exp so the PE's
                        # in-order queue never waits on the Scalar engine
                        if pending_o is not None:
                            pending_o(); pending_o = None
                        if g == 0 and pending is not None:
                            pending(); pending = None

                        def make_o(b, g, po, wt):
                            def emit_o():
                                # pairs of k-tiles as two concurrent
                                # column-tiled MMs (strips at PE cols 0/64
                                # -> psum partitions 0:64 / 64:128); the
                                # has_written clear is per written region,
                                # so EACH strip's first MM needs start=True
                                for j in range(4):
                                    kt = 4 * g + j
                                    odd = kt % 2
                                    nc.tensor.matmul(
                                        po[64 * odd:64 * odd + 64, :],
                                        v_sb[b][:, kt, :], wt[:, j, :],
                                        start=(kt <= 1), stop=(kt >= NKT - 2),
                                        tile_position=(0, 64 * odd),
                                        skip_group_check=True)
                            return emit_o
                        pending_o = make_o(b, g, po, wt)

                    def make_epilogue(qc, b, po):
                        def epilogue():
                            # evacuate po once; l (row 32) and the gated o
                            # DMA out raw -- the 1/l normalization and the
                            # Wo projection (1% of FLOPs) happen on host
                            posb = wpool.tile([33, 512], F32, tag="posb",
                                              name=f"posb{qc}{b}", bufs=2)
                            pob = wpool.tile([33, 512], F32, tag="pob",
                                             name=f"pob{qc}{b}", bufs=2)
                            nc.vector.tensor_copy(pob[:, :], po[64:97, :])
                            nc.vector.tensor_tensor(posb[:, :], po[0:33, :],
                                                    pob[:, :], op=ADD)
                            nc.sync.dma_start(out=lout[b, qc, :],
                                              in_=posb[32:33, :])
                            og = obpool.tile([32, 512], BF16, tag="og",
                                             name=f"og{qc}{b}")
                            nc.vector.tensor_tensor(
                                og[:, :], g_sb[b][:, qc * 512:(qc + 1) * 512],
                                posb[0:32, :], op=MULT)
                            nc.sync.dma_start(out=ogout[b, qc], in_=og[:, :])
                        return epilogue

                    pending = make_epilogue(qc, b, po)
            pending_o(); pending_o = None
            pending(); pending = None
    nc.compile()
    return nc


def _get_nc():
    if "nc" not in _CACHE:
        _CACHE["nc"] = _build()
    return _CACHE["nc"]


def kernel(q_x, kv_x, bias_mask, bias_pair, Wq, Wk, Wv, Wo, bo, Wg, bg):
    from concourse.bass_utils import run_bass_kernel_spmd

    nc = _get_nc()
    f32 = np.float32
    q_x = np.asarray(q_x, f32); kv_x = np.asarray(kv_x, f32)
    bias_mask = np.asarray(bias_mask, f32); bias_pair = np.asarray(bias_pair, f32)
    Wq = np.asarray(Wq, f32); Wk = np.asarray(Wk, f32); Wv = np.asarray(Wv, f32)
    Wo = np.asarray(Wo, f32); bo = np.asarray(bo, f32); Wg = np.asarray(Wg, f32)
    bg = np.asarray(bg, f32)

    import ml_dtypes
    _bf16 = ml_dtypes.bfloat16
    sD = 1.0 / math.sqrt(D)
    qxT_dev = np.ascontiguousarray(
        q_x.transpose(0, 2, 1).reshape(B, 2, 128, Q)).astype(_bf16)
    kvT_dev = np.ascontiguousarray(
        kv_x.transpose(0, 2, 1).reshape(B, 2, 128, K)).astype(_bf16)
    ebm = np.exp(bias_mask.reshape(B, NKT, 128).transpose(0, 2, 1))
    bm_dev = np.ascontiguousarray(ebm)
    bmb_dev = np.ascontiguousarray(ebm.astype(_bf16))

    def wsplit(W, h, scale=1.0):
        # [2, 128, D] view of (W_h * scale).T with W_h = W[h*D:(h+1)*D, :]
        return np.ascontiguousarray(
            (W[h * D:(h + 1) * D, :] * scale).T.reshape(2, 128, D)).astype(_bf16)

    def wrep(W, h, scale=1.0):
        # weight columns duplicated 4x -> M=128 matmul emits 4 replicas
        wt = wsplit(W, h, scale)                       # [2, 128, D]
        return np.ascontiguousarray(np.tile(wt, (1, 1, 4)))

    def wstrips(W, h):
        # strip j: W_h.T placed at columns 32j..32j+32, zeros elsewhere
        wt = wsplit(W, h)                              # [2, 128, D]
        out = np.zeros((4, 2, 128, 128), _bf16)
        for j in range(4):
            out[j, :, :, 32 * j:32 * j + 32] = wt
        return out

    def bias_tiles(h):
        # [qc, g, p, j, q']: strips 0,1 raw bias, strips 2,3 exp(bias);
        # k = g*512 + j*128 + p, q = qc*512 + q'
        bpT = bias_pair[0, h].T.reshape(4, 4, 128, 4, 512)   # g, j, p, qc, q'
        t = np.ascontiguousarray(bpT.transpose(3, 0, 2, 1, 4))
        for j in range(4):
            if not A_OF4[j]:
                t[:, :, :, j, :] = np.exp(t[:, :, :, j, :])
        return np.ascontiguousarray(t.astype(_bf16))

    def wpack(h):
        # [128, 1568]: wq 2x128 | wk 8x128 | id 128 | wv 64 | wg 64 | expbmb 32
        pk = np.zeros((128, 1568), _bf16)
        pk[:, 0:256] = wrep(Wq, h, sD).transpose(1, 0, 2).reshape(128, 256)
        ws = wstrips(Wk, h)                             # [4, 2, 128, 128]
        pk[:, 256:1280] = ws.transpose(2, 0, 1, 3).reshape(128, 1024)
        pk[:, 1280:1408] = np.eye(128, dtype=_bf16)
        pk[:, 1408:1472] = wsplit(Wv, h).transpose(1, 0, 2).reshape(128, 64)
        pk[:, 1472:1536] = wsplit(Wg, h).transpose(1, 0, 2).reshape(128, 64)
        pk[:, 1536:1568] = bmb_dev.transpose(1, 0, 2).reshape(128, 32)
        return pk

    in_maps = []
    for h in range(H):
        in_maps.append({
            "qxT": qxT_dev, "kvT": kvT_dev,
            "bpt": bias_tiles(h),
            "wpk": wpack(h),
            "expbm": bm_dev,
            "bgv": np.ascontiguousarray(0.5 * bg[h * D:(h + 1) * D, None]),
        })

    try:
        res = run_bass_kernel_spmd(nc, in_maps, core_ids=list(range(H)))
    except Exception:
        # rare transient accelerator fault -- one retry after a short pause
        import time as _time
        _time.sleep(5)
        res = run_bass_kernel_spmd(nc, in_maps, core_ids=list(range(H)))
    out = np.zeros((B, Q, C), f32)
    for h in range(H):
        og = res.results[h]["ogout"].astype(f32)         # [B, 4, 32, 512]
        l = res.results[h]["lout"].reshape(B, 1, Q)      # [B, 1, Q]
        og = og.transpose(0, 2, 1, 3).reshape(B, D, Q) / l
        # out.T partial = Wo_h @ (o*g/l)
        out += np.einsum("cd,bdq->bqc", Wo[:, h * D:(h + 1) * D], og,
                         optimize=True)
    out += bo
    return out


# revision 37
# speedup vs baseline: 1.1445x; 1.1445x over previous
"""Gated attention-with-pair-bias kernel for 8 Trainium2 NeuronCores.

Problem: B=2, Q=K=2048, C=256, H=8 heads, D=32 per head.
  q = (q_x @ Wq.T)/sqrt(D); k = kv_x @ Wk.T; v = kv_x @ Wv.T   (per head h)
  S = q @ k.T + bias_mask + bias_pair; w = softmax_k(S)
  o = (w @ v) * sigmoid(q_x @ Wg.T + bg); out = o @ Wo.T + bo

Sharding: one head per core (8 heads / 8 cores); each core handles both
batch elements.  Per-head output-projection partials are normalized by
the softmax denominator and summed on host (1/l commutes past Wo).

On-chip layout is "ST" (scores transposed): S.T tiles are [k->128
partitions, q->512 free].  The engine budget is dominated by the Act
(ScalarE) exp stream -- 8.4M score elements/core at 1 elem/cycle/lane is
a ~70us floor -- so every other engine is kept strictly below it:
  - score matmuls: D=32 contraction packed 4x with PE row tiling
    (tile_position); the 4 strip-MMs execute concurrently (~1 MM's time).
  - bias_pair add is split: strips 0,1 of each group get it via an
    identity-matmul PSUM pre-fill (PE has slack), strips 2,3 get it
    after the exp as a bf16 2x-mode DVE multiply with exp(bias_pair)
    precomputed on host: exp(s+b) = exp(s)*exp(b).
  - bias_mask folds into v (and into the appended ones-column that
    yields the softmax denominator from the o-matmul).
  - gates use tanh (same Act table-set as exp; sigmoid would force two
    ~2.7us table switches): sigmoid(z) = 0.5*tanh(z/2)+0.5.
  - everything HBM-side is bf16; bias_pair is restaged on host into
    fully-contiguous 512KB tiles so DMA runs at line rate.
"""

import math
import sys

sys.path.insert(0, "/opt/trn_rl_repo")

import numpy as np

H, D, B, Q, K, C = 8, 32, 2, 2048, 2048, 256
NQC = 4          # q chunks of 512
NKT = K // 128   # 16 k tiles
# which of the 4 strips per group take the identity-MM bias path (True)
# vs the post-exp multiply path (False)
A_OF4 = (True, True, False, False)

_CACHE = {}


def _build():
    import concourse.bacc as bacc
    import concourse.mybir as mybir
    from concourse.tile import TileContext

    F32 = mybir.dt.float32
    BF16 = mybir.dt.bfloat16
    EXP = mybir.ActivationFunctionType.Exp
    TANH = mybir.ActivationFunctionType.Tanh
    MULT = mybir.AluOpType.mult
    ADD = mybir.AluOpType.add

    nc = bacc.Bacc(None, target_bir_lowering=False)
    qxT = nc.dram_tensor("qxT", [B, 2, 128, Q], BF16, kind="ExternalInput")
    kvT = nc.dram_tensor("kvT", [B, 2, 128, K], BF16, kind="ExternalInput")
    # bias tiles: [qc, g, p, j, q']; j in {0,1}: raw bias (id-MM path),
    # j in {2,3}: exp(bias) (post-exp multiply path)
    bpt = nc.dram_tensor("bpt", [NQC, 4, 128, 4, 512], BF16, kind="ExternalInput")
    # all bf16 weights/constants in one blob -> one DMA on the sync queue
    # cols: wq 2x128 | wk 8x128 | id 128 | wv 2x32 | wg 2x32 | expbmb 2x16
    WPK = 1568
    wpk = nc.dram_tensor("wpk", [128, WPK], BF16, kind="ExternalInput")
    expbm = nc.dram_tensor("expbm", [B, 128, NKT], F32, kind="ExternalInput")
    bgv = nc.dram_tensor("bgv", [D, 1], F32, kind="ExternalInput")
    ogout = nc.dram_tensor("ogout", [B, NQC, 32, 512], BF16, kind="ExternalOutput")
    lout = nc.dram_tensor("lout", [B, NQC, 512], F32, kind="ExternalOutput")

    with TileContext(nc) as tc:
        with (
            tc.tile_pool(name="ld", bufs=1) as ld,
            tc.tile_pool(name="pers", bufs=1) as pers,
            tc.tile_pool(name="bp", bufs=8) as bppool,
            tc.tile_pool(name="wp", bufs=3) as wpool,
            tc.tile_pool(name="ob", bufs=2) as obpool,
            tc.tile_pool(name="ps_sc", bufs=2, space="PSUM") as ps_sc,
            tc.tile_pool(name="ps_o", bufs=2, space="PSUM") as ps_o,
            tc.tile_pool(name="ps_m", bufs=2, space="PSUM") as ps_m,
        ):
            # ---- one packed-weight DMA, then per-batch activations, so
            # the first projection matmuls start as early as possible ----
            # warm the Act table set (exp/tanh) off the critical path
            dummy = pers.tile([1, 2], F32, name="dummy")
            nc.vector.memset(dummy[:, :], 0.0)
            nc.scalar.activation(dummy[:, 0:1], dummy[:, 1:2], EXP)

            wpk_sb = pers.tile([128, 1568], BF16, name="wpk_sb")
            nc.sync.dma_start(out=wpk_sb[:, :], in_=wpk[:, :])
            wq_sb = [wpk_sb[:, ch * 128:(ch + 1) * 128] for ch in range(2)]
            wk_sb = [[wpk_sb[:, 256 + (j * 2 + ch) * 128:256 + (j * 2 + ch + 1) * 128]
                      for ch in range(2)] for j in range(4)]
            id_sb = wpk_sb[:, 1280:1408]
            wv_sb = [wpk_sb[:, 1408 + ch * 32:1408 + (ch + 1) * 32] for ch in range(2)]
            wg_sb = [wpk_sb[:, 1472 + ch * 32:1472 + (ch + 1) * 32] for ch in range(2)]
            bmc_sb = [wpk_sb[:, 1536 + b * 16:1536 + (b + 1) * 16] for b in range(B)]

            qx_all, kv_all = [], []

            def load_acts(b):
                for ch in range(2):
                    t = ld.tile([128, Q], BF16, name=f"qx{b}{ch}", tag=f"qx{b}{ch}")
                    nc.sync.dma_start(out=t[:, :], in_=qxT[b, ch, :, :])
                    qx_all.append(t)
                    t = ld.tile([128, K], BF16, name=f"kv{b}{ch}", tag=f"kv{b}{ch}")
                    nc.sync.dma_start(out=t[:, :], in_=kvT[b, ch, :, :])
                    kv_all.append(t)

            load_acts(0)
            bm_sb = []
            for b in range(B):
                t = pers.tile([128, NKT], F32, name=f"bm_sb{b}")
                nc.sync.dma_start(out=t[:, :], in_=expbm[b, :, :])
                bm_sb.append(t)
            bg_sb = pers.tile([D, 1], F32, name="bg_sb")
            nc.sync.dma_start(out=bg_sb[:, :], in_=bgv[:, :])
            # qc0's bias tiles BEFORE the b=1 activations: the first exp
            # is gated on bias arrival, b=1 isn't needed until ~35us
            bts0 = []
            for g in range(4):
                t = bppool.tile([128, 4, 512], BF16, tag="bp", name=f"bp0_{g}")
                nc.sync.dma_start(out=t[:, :, :], in_=bpt[0, g])
                bts0.append(t)
            load_acts(1)

            # ---- per-batch projections ----
            # b=0 emits up front; b=1 is deferred into the qc0/b0 compute
            # window so its DMA waits hide under main-loop work.  b=1's
            # v-projection is deferred further, in 4-ktile chunks spread
            # across the (qc0,b1) group bodies, so the in-order PE queue
            # reaches b1's first score MMs (which gate the Act exp
            # stream) ~7us earlier; each chunk lands just ahead of the
            # trailing o-MMs that consume it.
            qT_rep, kT_sb, v_sb, g_sb = [], [], [], []
            emit_v = []

            def emit_prologue(b):
                qx_b = qx_all[2 * b:2 * b + 2]
                kv_b = kv_all[2 * b:2 * b + 2]
                qT = pers.tile([128, Q], BF16, name=f"qT{b}")
                kT = pers.tile([128, 512], BF16, name=f"kT{b}")
                # [d 0:32 | expbm 32 | zeros 33:64]: 64-wide so pairs of
                # k-tiles run as two concurrent column-tiled o-matmuls
                vt = pers.tile([128, NKT, 64], BF16, name=f"v{b}")
                nc.vector.memset(vt[:, :, :], 0.0)
                gs = pers.tile([32, Q], F32, name=f"g{b}")
                qT_rep.append(qT); kT_sb.append(kT); v_sb.append(vt); g_sb.append(gs)

                # q.T replicated into 4 partition strips via 4x-duplicated
                # weight columns (host-prepared) -> plain M=128 matmuls
                for qc in range(NQC):
                    ps = ps_sc.tile([128, 512], F32, tag="sc", name=f"pq{b}{qc}",
                                    padded_shape=[128, 1024])
                    for ch in range(2):
                        nc.tensor.matmul(
                            ps[:, :], wq_sb[ch][:, :],
                            qx_b[ch][:, qc * 512:(qc + 1) * 512],
                            start=(ch == 0), stop=(ch == 1))
                    nc.vector.tensor_copy(qT[:, qc * 512:(qc + 1) * 512], ps[:, :])

                # k.T in strip layout (strip j holds k tiles {4g+j : g}) via
                # zero-padded strip weights accumulating into one bank
                ps = ps_sc.tile([128, 512], F32, tag="sc", name=f"pk{b}",
                                padded_shape=[128, 1024])
                for j in range(4):
                    kv_r = [t.rearrange("p (g j i) -> p j g i", g=4, i=128)
                            for t in kv_b]
                    for ch in range(2):
                        nc.tensor.matmul(
                            ps[:, :], wk_sb[j][ch][:, :], kv_r[ch][:, j],
                            start=(j == 0 and ch == 0), stop=(j == 3 and ch == 1))
                nc.vector.tensor_copy(kT[:, :], ps[:, :])

                # gate = sigmoid(Wg_h @ qx.T + bg) = 0.5*tanh((Wg@qx+bg)/2)+0.5
                # tanh shares the exp table-set; the +0.5 scale/offset is DVE
                for qc in range(NQC):
                    pg = ps_m.tile([32, 512], F32, tag="m", name=f"pg{b}{qc}",
                                   padded_shape=[128, 512])
                    for ch in range(2):
                        nc.tensor.matmul(
                            pg[:, :], wg_sb[ch][:, :],
                            qx_b[ch][:, qc * 512:(qc + 1) * 512],
                            start=(ch == 0), stop=(ch == 1))
                    nc.scalar.activation(gs[:, qc * 512:(qc + 1) * 512], pg[:, :],
                                         TANH, bias=bg_sb[:, :], scale=0.5)
                nc.vector.tensor_scalar(gs[:, :], gs[:, :], 0.5, 0.5,
                                        op0=MULT, op1=mybir.AluOpType.add)

                # v in [k-partitions, d] layout; the bias_mask fold
                # (exp(bm) per k row) rides the PSUM evacuation, and the
                # appended ones-column IS exp(bm).  Per-kt psum tiles keep
                # PE-writes and DVE-reads on separate banks (pool
                # rotation); the ones-column goes through a contiguous
                # SBUF staging tile so every vt write is an engine op
                # with solid dependency tracking.
                def emit_v_chunk(b, vt, c):
                    kv_b = kv_all[2 * b:2 * b + 2]
                    for kt in range(4 * c, 4 * c + 4):
                        pv = ps_m.tile([128, 32], F32, tag="m",
                                       name=f"pv{b}{kt}",
                                       padded_shape=[128, 512])
                        for ch in range(2):
                            nc.tensor.matmul(
                                pv[:, :], kv_b[ch][:, kt * 128:(kt + 1) * 128],
                                wv_sb[ch][:, :], start=(ch == 0), stop=(ch == 1))
                        nc.vector.tensor_scalar_mul(vt[:, kt, 0:32], pv[:, :],
                                                    bm_sb[b][:, kt:kt + 1])
                    nc.vector.tensor_copy(vt[:, 4 * c:4 * c + 4, 32],
                                          bmc_sb[b][:, 4 * c:4 * c + 4])

                if b == 0:
                    for c in range(4):
                        emit_v_chunk(0, vt, c)
                else:
                    emit_v[:] = [lambda c=c: emit_v_chunk(1, vt, c)
                                 for c in range(4)]

            emit_prologue(0)

            # ---- main attention loop ----
            pending = None
            pending_o = None
            for qc in range(NQC):
                if qc == 0:
                    bts = bts0
                else:
                    bts = []
                    for g in range(4):
                        # [128, 4, 512]: one group's (4 k-tiles) bias for
                        # this q chunk, host-restaged fully contiguous
                        t = bppool.tile([128, 4, 512], BF16, tag="bp",
                                        name=f"bp{qc}_{g}")
                        nc.sync.dma_start(out=t[:, :, :], in_=bpt[qc, g])
                        bts.append(t)
                for b in range(B):
                    if qc == 0 and b == 1:
                        emit_prologue(1)
                    po = ps_o.tile([128, 512], F32, tag="o", name=f"po{qc}{b}")
                    for g in range(4):
                        if qc == 0 and b == 1 and g >= 1 and emit_v:
                            emit_v.pop(0)()
                            if g == 3 and emit_v:
                                emit_v.pop(0)()
                        # strips 0,1 (psA): bias via identity-MM PSUM
                        # pre-fill; strips 2,3 (psB): plain scores, bias
                        # multiplied in after the exp
                        psA = ps_sc.tile([128, 2, 512], F32, tag="sc",
                                         name=f"sA{qc}{b}{g}")
                        psB = ps_sc.tile([128, 2, 512], F32, tag="sc",
                                         name=f"sB{qc}{b}{g}")
                        for j in range(4):
                            if A_OF4[j]:
                                ps, jj = (psA, j) if j < 2 else (psB, j - 2)
                                nc.tensor.matmul(
                                    ps[:, jj, :], id_sb[:, :], bts[g][:, j, :],
                                    start=True, stop=False)
                        for j in range(4):
                            ps, jj = (psA, j) if j < 2 else (psB, j - 2)
                            nc.tensor.matmul(
                                ps[:, jj, :],
                                kT_sb[b][32 * j:32 * j + 32, g * 128:(g + 1) * 128],
                                qT_rep[b][32 * j:32 * j + 32, qc * 512:(qc + 1) * 512],
                                start=not A_OF4[j], stop=True,
                                tile_position=(32 * j, 0))
                        wt = wpool.tile([128, 4, 512], BF16, tag="w",
                                        name=f"w{qc}{b}{g}")
                        for ti, ps in ((0, psA), (1, psB)):
                            sl = slice(2 * ti, 2 * ti + 2)
                            if A_OF4[2 * ti] and A_OF4[2 * ti + 1]:
                                nc.scalar.activation(wt[:, sl, :], ps[:, :, :], EXP)
                            else:
                                # exp(s+b) = exp(s)*exp(b): bf16 2x DVE mult
                                wtmp = wpool.tile([128, 2, 512], BF16, tag="wtmp",
                                                  name=f"wtmp{qc}{b}{g}{ti}", bufs=2)
                                nc.scalar.activation(wtmp[:, :, :], ps[:, :, :], EXP)
                                nc.vector.tensor_tensor(wt[:, sl, :], wtmp[:, :, :],
                                                        bts[g][:, sl, :], op=MULT)
                        # o-MMs run one group behind their exp so the PE's
                        # in-order queue never waits on the Scalar engine
                        if pending_o is not None:
                            pending_o(); pending_o = None
                        if g == 0 and pending is not None:
                            pending(); pending = None

                        def make_o(b, g, po, wt):
                            def emit_o():
                                # pairs of k-tiles as two concurrent
                                # column-tiled MMs (strips at PE cols 0/64
                                # -> psum partitions 0:64 / 64:128); the
                                # has_written clear is per written region,
                                # so EACH strip's first MM needs start=True
                                for j in range(4):
                                    kt = 4 * g + j
                                    odd = kt % 2
                                    nc.tensor.matmul(
                                        po[64 * odd:64 * odd + 64, :],
                                        v_sb[b][:, kt, :], wt[:, j, :],
                                        start=(kt <= 1), stop=(kt >= NKT - 2),
                                        tile_position=(0, 64 * odd),
                                        skip_group_check=True)
                            return emit_o
                        pending_o = make_o(b, g, po, wt)

                    def make_epilogue(qc, b, po):
                        def epilogue():
                            # evacuate po once; l (row 32) and the gated o
                            # DMA out raw -- the 1/l normalization and the
                            # Wo projection (1% of FLOPs) happen on host
                            posb = wpool.tile([33, 512], F32, tag="posb",
                                              name=f"posb{qc}{b}", bufs=2)
                            pob = wpool.tile([33, 512], F32, tag="pob",
                                             name=f"pob{qc}{b}", bufs=2)
                            nc.vector.tensor_copy(pob[:, :], po[64:97, :])
                            nc.vector.tensor_tensor(posb[:, :], po[0:33, :],
                                                    pob[:, :], op=ADD)
                            nc.sync.dma_start(out=lout[b, qc, :],
                                              in_=posb[32:33, :])
                            og = obpool.tile([32, 512], BF16, tag="og",
                                             name=f"og{qc}{b}")
                            nc.vector.tensor_tensor(
                                og[:, :], g_sb[b][:, qc * 512:(qc + 1) * 512],
                                posb[0:32, :], op=MULT)
                            nc.sync.dma_start(out=ogout[b, qc], in_=og[:, :])
                        return epilogue

                    pending = make_epilogue(qc, b, po)
            pending_o(); pending_o = None
            pending(); pending = None
    nc.compile()
    return nc


def _get_nc():
    if "nc" not in _CACHE:
        _CACHE["nc"] = _build()
    return _CACHE["nc"]


def kernel(q_x, kv_x, bias_mask, bias_pair, Wq, Wk, Wv, Wo, bo, Wg, bg):
    from concourse.bass_utils import run_bass_kernel_spmd

    nc = _get_nc()
    f32 = np.float32
    q_x = np.asarray(q_x, f32); kv_x = np.asarray(kv_x, f32)
    bias_mask = np.asarray(bias_mask, f32); bias_pair = np.asarray(bias_pair, f32)
    Wq = np.asarray(Wq, f32); Wk = np.asarray(Wk, f32); Wv = np.asarray(Wv, f32)
    Wo = np.asarray(Wo, f32); bo = np.asarray(bo, f32); Wg = np.asarray(Wg, f32)
    bg = np.asarray(bg, f32)

    import ml_dtypes
    _bf16 = ml_dtypes.bfloat16
    sD = 1.0 / math.sqrt(D)
    qxT_dev = np.ascontiguousarray(
        q_x.transpose(0, 2, 1).reshape(B, 2, 128, Q)).astype(_bf16)
    kvT_dev = np.ascontiguousarray(
        kv_x.transpose(0, 2, 1).reshape(B, 2, 128, K)).astype(_bf16)
    ebm = np.exp(bias_mask.reshape(B, NKT, 128).transpose(0, 2, 1))
    bm_dev = np.ascontiguousarray(ebm)
    bmb_dev = np.ascontiguousarray(ebm.astype(_bf16))

    def wsplit(W, h, scale=1.0):
        # [2, 128, D] view of (W_h * scale).T with W_h = W[h*D:(h+1)*D, :]
        return np.ascontiguousarray(
            (W[h * D:(h + 1) * D, :] * scale).T.reshape(2, 128, D)).astype(_bf16)

    def wrep(W, h, scale=1.0):
        # weight columns duplicated 4x -> M=128 matmul emits 4 replicas
        wt = wsplit(W, h, scale)                       # [2, 128, D]
        return np.ascontiguousarray(np.tile(wt, (1, 1, 4)))

    def wstrips(W, h):
        # strip j: W_h.T placed at columns 32j..32j+32, zeros elsewhere
        wt = wsplit(W, h)                              # [2, 128, D]
        out = np.zeros((4, 2, 128, 128), _bf16)
        for j in range(4):
            out[j, :, :, 32 * j:32 * j + 32] = wt
        return out

    def bias_tiles(h):
        # [qc, g, p, j, q']: strips 0,1 raw bias, strips 2,3 exp(bias);
        # k = g*512 + j*128 + p, q = qc*512 + q'
        bpT = bias_pair[0, h].T.reshape(4, 4, 128, 4, 512)   # g, j, p, qc, q'
        t = np.ascontiguousarray(bpT.transpose(3, 0, 2, 1, 4))
        for j in range(4):
            if not A_OF4[j]:
                t[:, :, :, j, :] = np.exp(t[:, :, :, j, :])
        return np.ascontiguousarray(t.astype(_bf16))

    def wpack(h):
        # [128, 1568]: wq 2x128 | wk 8x128 | id 128 | wv 64 | wg 64 | expbmb 32
        pk = np.zeros((128, 1568), _bf16)
        pk[:, 0:256] = wrep(Wq, h, sD).transpose(1, 0, 2).reshape(128, 256)
        ws = wstrips(Wk, h)                             # [4, 2, 128, 128]
        pk[:, 256:1280] = ws.transpose(2, 0, 1, 3).reshape(128, 1024)
        pk[:, 1280:1408] = np.eye(128, dtype=_bf16)
        pk[:, 1408:1472] = wsplit(Wv, h).transpose(1, 0, 2).reshape(128, 64)
        pk[:, 1472:1536] = wsplit(Wg, h).transpose(1, 0, 2).reshape(128, 64)
        pk[:, 1536:1568] = bmb_dev.transpose(1, 0, 2).reshape(128, 32)
        return pk

    in_maps = []
    for h in range(H):
        in_maps.append({
            "qxT": qxT_dev, "kvT": kvT_dev,
            "bpt": bias_tiles(h),
            "wpk": wpack(h),
            "expbm": bm_dev,
            "bgv": np.ascontiguousarray(0.5 * bg[h * D:(h + 1) * D, None]),
        })

    try:
        res = run_bass_kernel_spmd(nc, in_maps, core_ids=list(range(H)))
    except Exception:
        # rare transient accelerator fault -- one retry after a short pause
        import time as _time
        _time.sleep(5)
        res = run_bass_kernel_spmd(nc, in_maps, core_ids=list(range(H)))
    out = np.zeros((B, Q, C), f32)
    for h in range(H):
        og = res.results[h]["ogout"].astype(f32)         # [B, 4, 32, 512]
        l = res.results[h]["lout"].reshape(B, 1, Q)      # [B, 1, Q]
        og = og.transpose(0, 2, 1, 3).reshape(B, D, Q) / l
        # out.T partial = Wo_h @ (o*g/l)
        out += np.einsum("cd,bdq->bqc", Wo[:, h * D:(h + 1) * D], og,
                         optimize=True)
    out += bo
    return out
